# revision 1
# baseline (speedup 1.0000x reference)
"""ClassBalancedSupConLoss on 8 TRN2 NeuronCores (Bass/Tile).

Math (reference semantics, reorganized for hardware):
  - All embeddings are unit-norm, so s_ij = e_i . e_j <= 1 and s_ii ~= 1.
    Use a FIXED logsumexp shift m = 1:
        LSE_i = inv_t_i * 1 + log( sum_j exp(inv_t_i * (s_ij - 1)) )
    The self term is excluded by subtracting exp(inv_t*(s_ii-1)) where
    s_ii is computed ON DEVICE from the same rounded operands (bitwise
    identical to the self term inside the big sum, so the cancellation
    is exact even though matmul-input rounding makes s_ii != 1).
  - Batch and bank are sorted by class on the host, so the same-class
    column set of any anchor is one contiguous segment.  Bank same-class
    exclusion = (total exp sum) - (own-class segment exp sum); positives
    = (own-class raw-logit segment sum - s_ii) / pos_cnt.
  - Anchors (batch rows) are sharded 256/core across 8 cores; every core
    holds full embT/bankT replicas.  Per-anchor losses are DMA'd out;
    the final masked mean over 2048 anchors is a host-side reduction.

Engine structure per core (2 anchor tiles x [128 anchors]):
  - PE: S chunks [128, 512] into rotating [128, 2048] PSUM tiles
    (2 tiles x 4 banks).  bf16 inputs (fast FWL weight loads, 1 cyc/row).
  - ACT: one Exp pass per 2048-col PSUM chunk with accum_out row-sums;
    exp calls are SPLIT at class-segment boundaries, so per-class bank
    exp sums fall out of the per-call accumulators directly.
  - DVE: raw-logit segment reductions for positives + tiny epilogue.

SPMD: one program for all 8 cores.  Anything core-dependent (the anchor
slice, per-anchor temperature vectors, one-hot class rows) is passed as
per-core DATA; program constants (class segment boundaries) are global.
"""

import os
import numpy as np

import concourse.bass as bass  # noqa: F401
from concourse import bacc
import concourse.mybir as mybir
import concourse.tile as tile
from concourse.bass_utils import run_bass_kernel_spmd

B, D, M, C = 2048, 128, 16384, 3
NCORES = 8
APC = B // NCORES          # anchors per core = 256
NT = APC // 128            # anchor tiles per core = 2
CH = 512                   # matmul free chunk (one PSUM bank)
W = 2048                   # big PSUM chunk (4 banks) = one ACT Exp pass
NBK = M // W               # 8 bank pieces of [128, 2048]
BASE_TEMP = 0.07

F32 = mybir.dt.float32
AF = mybir.ActivationFunctionType
ALU = mybir.AluOpType
AX = mybir.AxisListType

# "bf16": matmul inputs bf16 (fast path; ~1e-3 logit rounding)
# "f32r": fp32 bits, PE rounds mantissa (slow LDWEIGHTS, ~4x PE time)
# "f32" : full fp32 matmul (4 cyc/row)
MM_MODE = os.environ.get("SUPCON_MM_MODE", "bf16")

LAST_EXEC_TIME_NS = None   # set by kernel() when SUPCON_TRACE=1


def _install_trace_shim():
    """Register the NTFF profile hook that this image's antenv lacks.

    Mirrors trn_agent_boot's _ntff_profile_via_ctypes: drives NRT
    profiling via the injected libaxon_pjrt.so.  Only used for local
    perf iteration (SUPCON_TRACE=1); the plain execution path never
    needs it.
    """
    import sys
    import types
    import ctypes
    import contextlib

    try:
        from antenv.axon_hooks import get_axon_ntff_profile_hook  # noqa: F401
        return True  # real module exists
    except ImportError:
        pass

    so_path = "/opt/axon/libaxon_pjrt.so"
    if not os.path.exists(so_path):
        return False
    lib = ctypes.CDLL(so_path)
    if not hasattr(lib, "axon_start_nrt_profile"):
        return False
    lib.axon_start_nrt_profile.argtypes = [
        ctypes.POINTER(ctypes.c_int64),
        ctypes.c_size_t,
    ]
    lib.axon_start_nrt_profile.restype = ctypes.c_int64
    lib.axon_stop_nrt_profile.argtypes = [ctypes.c_char_p]
    lib.axon_stop_nrt_profile.restype = ctypes.c_int64

    @contextlib.contextmanager
    def _hook(output_dir, device_ids):
        import jax

        jax.devices()
        if device_ids:
            ids = (ctypes.c_int64 * len(device_ids))(*device_ids)
            rc = lib.axon_start_nrt_profile(ids, len(device_ids))
        else:
            rc = lib.axon_start_nrt_profile(None, 0)
        if rc != 0:
            raise RuntimeError(f"axon_start_nrt_profile rc={rc}")
        try:
            yield
        finally:
            n = lib.axon_stop_nrt_profile(str(output_dir).encode())
            print(f"profile: {n} file(s) written to {output_dir}", file=sys.stderr)

    _state = {"hook": _hook}
    mod = types.ModuleType("antenv.axon_hooks")
    mod.get_axon_ntff_profile_hook = lambda: _state["hook"]
    mod.set_axon_ntff_profile_hook = lambda h: _state.update(hook=h)
    sys.modules["antenv.axon_hooks"] = mod
    import antenv

    antenv.axon_hooks = mod

    # skip the artifact upload (no bucket access needed for local iteration)
    import concourse.bass_utils as bu

    bu.upload_artifacts = lambda tmpdir: tmpdir
    return True


def _bank_subranges(mk_b1, mk_b2):
    """Split [0, M) at big-chunk multiples AND class boundaries.

    Returns (subs, i1, i2): subs = list of (start, end); i1/i2 = first
    subrange index at/after mk_b1/mk_b2 (class-segment column ranges in
    the per-subrange accumulator tile are then [0,i1), [i1,i2), [i2,n)).
    """
    cuts = sorted({c * W for c in range(NBK + 1)} | {mk_b1, mk_b2})
    subs = [(cuts[i], cuts[i + 1]) for i in range(len(cuts) - 1)]
    i1 = sum(1 for s, _ in subs if s < mk_b1)
    i2 = sum(1 for s, _ in subs if s < mk_b2)
    return subs, i1, i2


def _build(bb_b1, bb_b2, mk_b1, mk_b2, mm_mode):
    import ml_dtypes  # noqa: F401  (bf16 numpy dtype registration)

    if mm_mode == "bf16":
        in_dt = mybir.dt.bfloat16
    elif mm_mode == "f32":
        in_dt = F32
    else:
        in_dt = mybir.dt.float32r

    nc = bacc.Bacc()
    embT_d = nc.declare_dram_parameter("embT", [D, B], in_dt, isOutput=False)
    anchT_d = nc.declare_dram_parameter("anchT", [D, APC + C], in_dt, isOutput=False)
    bankT_d = nc.declare_dram_parameter("bankT", [D, M], in_dt, isOutput=False)
    subs, i1, i2 = _bank_subranges(mk_b1, mk_b2)
    NK = len(subs)
    # one packed small-vector input: [invt | ninvt | invpc | coefv | oneh |
    # incl | eye] along columns -- a single DMA instead of seven
    NV = NT * (4 + C + NK) + 128
    vecs_d = nc.declare_dram_parameter("vecs", [128, NV], F32, isOutput=False)
    oout_d = nc.declare_dram_parameter("oout", [128, 2 * NT], F32, isOutput=True)

    with tile.TileContext(nc) as tc:
        with (
            tc.tile_pool(name="big", bufs=1) as bigp,
            tc.tile_pool(name="sm", bufs=1) as smp,
            tc.tile_pool(name="ps", bufs=2, space="PSUM") as psp,
        ):
            anch_t = bigp.tile([D, APC + C], in_dt, tag="anchT")
            vecs_t = smp.tile([128, NV], F32, tag="vecs")
            # garbage-operand warmup tiles (never written: no DMA dependency,
            # so the PE can start immediately and open the HAM clock gate)
            junkw_t = bigp.tile([128, 128], in_dt, tag="junkw")
            junkx_t = bigp.tile([128, CH], in_dt, tag="junkx")
            o = [0]
            def vslice(w):
                a = o[0]; o[0] += w
                return vecs_t[:, a:a + w]
            invt_t = vslice(NT)
            ninvt_t = vslice(NT)
            invpc_t = vslice(NT)
            coefv_t = vslice(NT)
            oneh_t = vslice(NT * C)
            incl_t = vslice(NT * NK)
            eye_t = vslice(128)
            # both HWDGE queues (sync + scalar), pieces ordered by the time
            # the chunk stream consumes them; vecs first (unblocks the ACT
            # warmup), emb at quarter grain so the first bb matmuls start
            # as soon as the first 512 columns land
            emb_t = bigp.tile([D, B], in_dt, tag="embT")
            bank_ts = [bigp.tile([D, W], in_dt, tag=f"bank{j}", name=f"bank{j}")
                       for j in range(NBK)]
            H = B // 2
            Q = B // 4
            nc.sync.dma_start(out=vecs_t[:], in_=vecs_d[:])
            nc.scalar.dma_start(out=anch_t[:], in_=anchT_d[:])
            nc.sync.dma_start(out=emb_t[:, 0:Q], in_=embT_d[:, 0:Q])
            nc.scalar.dma_start(out=emb_t[:, Q:H], in_=embT_d[:, Q:H])
            nc.sync.dma_start(out=emb_t[:, H:H + Q], in_=embT_d[:, H:H + Q])
            nc.scalar.dma_start(out=emb_t[:, H + Q:B], in_=embT_d[:, H + Q:B])
            nc.sync.dma_start(out=bank_ts[0][:, 0:H], in_=bankT_d[:, 0:H])
            nc.scalar.dma_start(out=bank_ts[0][:, H:W], in_=bankT_d[:, H:W])
            nc.sync.dma_start(out=bank_ts[1][:, 0:H], in_=bankT_d[:, W:W + H])
            nc.scalar.dma_start(out=bank_ts[1][:, H:W], in_=bankT_d[:, W + H:2 * W])
            for j in range(2, NBK):
                eng = nc.sync if j % 2 == 0 else nc.scalar
                eng.dma_start(out=bank_ts[j][:], in_=bankT_d[:, j * W:(j + 1) * W])

            oout_t = smp.tile([128, 2 * NT], F32, tag="oout")
            scr_t = smp.tile([128, W], F32, tag="scrshared")
            sdiag = [smp.tile([128, 1], F32, tag=f"sdiag{t}", name=f"sdiag{t}") for t in range(NT)]
            selfe = [smp.tile([128, 1], F32, tag=f"selfe{t}", name=f"selfe{t}") for t in range(NT)]
            eyemul = smp.tile([128, 128], F32, tag="eyemul")
            warm = smp.tile([128, 1], F32, tag="warm")
            bbsum = [smp.tile([128, 1], F32, tag=f"bbsum{t}", name=f"bbsum{t}") for t in range(NT)]
            raw3 = [smp.tile([128, C], F32, tag=f"raw3{t}", name=f"raw3{t}") for t in range(NT)]
            esum = [smp.tile([128, NK], F32, tag=f"esum{t}", name=f"esum{t}") for t in range(NT)]

            # pull the Exp table load off the critical path
            nc.scalar.activation(warm[:], eye_t[:, 0:1], AF.Exp)

            def anch(t):
                return anch_t[:, t * 128:(t + 1) * 128]

            # ~4.3us of contiguous PE activity before the DMAs land: HAM
            # un-throttles (1.2 -> 2.4 GHz) before the real stream begins
            nc.vector.memset(junkw_t[:], 0.0)
            nc.vector.memset(junkx_t[:], 0.0)
            warm_ps = psp.tile([128, W], F32, tag="chunk", name="warm_ps")
            for w in range(8):
                nc.tensor.matmul(
                    warm_ps[:, (w % 4) * CH:((w % 4) + 1) * CH],
                    junkw_t[:], junkx_t[:], start=True, stop=True,
                )

            # ---- prelude: self-similarity blocks (diag -> s_ii) ----
            pre_ps = psp.tile([128, W], F32, tag="chunk", name="pre_ps")
            for t in range(NT):
                nc.tensor.matmul(
                    pre_ps[:, t * 128:(t + 1) * 128], anch(t), anch(t),
                    start=True, stop=True,
                )
            # raw positive segment sums as matmuls: raw3[i, c] = e_i . g_c
            # (g_c = class-sum embedding vectors, 3 extra anchT columns) --
            # keeps the [128, B] raw reductions off the DVE/PSUM critical path
            for t in range(NT):
                nc.tensor.matmul(
                    pre_ps[:, 256 + t * C:256 + (t + 1) * C], anch(t),
                    anch_t[:, APC:APC + C], start=True, stop=True,
                )
            for t in range(NT):
                nc.vector.tensor_mul(eyemul[:], pre_ps[:, t * 128:(t + 1) * 128], eye_t[:])
                nc.vector.reduce_sum(sdiag[t][:], eyemul[:], axis=AX.X)
                nc.vector.tensor_copy(out=raw3[t][:], in_=pre_ps[:, 256 + t * C:256 + (t + 1) * C])
                nc.scalar.activation(
                    selfe[t][:], sdiag[t][:], AF.Exp,
                    bias=ninvt_t[:, t:t + 1], scale=invt_t[:, t:t + 1],
                )

            by_chunk = {}
            for k, (s, e) in enumerate(subs):
                by_chunk.setdefault(s // W, []).append((s, e, k))

            scrNK = [smp.tile([128, NK], F32, tag=f"scrNK{t}", name=f"scrNK{t}") for t in range(NT)]
            scrC = [smp.tile([128, C], F32, tag=f"scrC{t}", name=f"scrC{t}") for t in range(NT)]

            def epi_early(t):
                """olin = coefv*invt*(1 - pos): depends only on prelude
                outputs (raw3/sdiag), so it runs during the exp stream."""
                own_r = smp.tile([128, 1], F32, tag=f"ownr{t}", name=f"ownr{t}")
                pos = smp.tile([128, 1], F32, tag=f"pos{t}", name=f"pos{t}")
                w1 = smp.tile([128, 1], F32, tag=f"w1{t}", name=f"w1{t}")
                p1 = smp.tile([128, 1], F32, tag=f"p1{t}", name=f"p1{t}")
                nc.vector.tensor_mul(scrC[t][:], raw3[t][:], oneh_t[:, t * C:(t + 1) * C])
                nc.vector.reduce_sum(own_r[:], scrC[t][:], axis=AX.X)
                nc.vector.scalar_tensor_tensor(
                    out=pos[:], in0=own_r[:], scalar=sdiag[t][:], in1=invpc_t[:, t:t + 1],
                    op0=ALU.subtract, op1=ALU.mult,
                )
                nc.vector.scalar_tensor_tensor(
                    out=w1[:], in0=pos[:], scalar=-1.0, in1=invt_t[:, t:t + 1],
                    op0=ALU.mult, op1=ALU.mult,
                )
                nc.vector.scalar_tensor_tensor(
                    out=oout_t[:, NT + t:NT + t + 1], in0=w1[:], scalar=invt_t[:, t:t + 1],
                    in1=coefv_t[:, t:t + 1], op0=ALU.add, op1=ALU.mult,
                )
                return p1

            p1s = {}

            def epilogue(t):
                """den = (bbsum - selfe) + sum_k esum_k * incl_k -- the only
                work that must trail the exp stream."""
                nc.vector.tensor_mul(scrNK[t][:], esum[t][:], incl_t[:, t * NK:(t + 1) * NK])
                nc.vector.reduce_sum(oout_t[:, t:t + 1], scrNK[t][:], axis=AX.X)
                nc.vector.tensor_add(oout_t[:, t:t + 1], oout_t[:, t:t + 1], p1s[t][:])

            def emit_bb(t):
                ps = psp.tile([128, W], F32, tag="chunk", name="bb_ps")
                for q in range(W // CH):
                    nc.tensor.matmul(
                        ps[:, q * CH:(q + 1) * CH], anch(t),
                        emb_t[:, q * CH:(q + 1) * CH],
                        start=True, stop=True,
                    )
                nc.scalar.activation(
                    scr_t[:], ps[:], AF.Exp,
                    bias=ninvt_t[:, t:t + 1], scale=invt_t[:, t:t + 1],
                    accum_out=bbsum[t][:],
                )

            def emit_bank(t, j):
                ps = psp.tile([128, W], F32, tag="chunk", name="bk_ps")
                for q in range(W // CH):
                    nc.tensor.matmul(
                        ps[:, q * CH:(q + 1) * CH], anch(t),
                        bank_ts[j][:, q * CH:(q + 1) * CH],
                        start=True, stop=True,
                    )
                for (s, e, k) in by_chunk[j]:
                    a, b = s - j * W, e - j * W
                    nc.scalar.activation(
                        scr_t[:, a:b], ps[:, a:b], AF.Exp,
                        bias=ninvt_t[:, t:t + 1], scale=invt_t[:, t:t + 1],
                        accum_out=esum[t][:, k:k + 1],
                    )

            # all of t0 (its DVE-only epilogue overlaps t1's stream); t1's
            # first chunk emitted before t0's last so PE never drains
            for t in range(NT):
                p1s[t] = epi_early(t)
            emit_bb(0)
            nc.vector.tensor_sub(p1s[0][:], bbsum[0][:], selfe[0][:])
            for j in range(NBK - 1):
                emit_bank(0, j)
            emit_bb(1)
            nc.vector.tensor_sub(p1s[1][:], bbsum[1][:], selfe[1][:])
            emit_bank(0, NBK - 1)
            epilogue(0)
            for j in range(NBK):
                emit_bank(1, j)
            epilogue(1)

            nc.sync.dma_start(out=oout_d[:], in_=oout_t[:])

    nc.compile()
    return nc


def _per_core_cols(vec, core):
    """[B] host vector -> [128, NT] tile for one core (col t, partition p)."""
    sl = vec[core * APC:(core + 1) * APC]
    return np.ascontiguousarray(sl.reshape(NT, 128).T).astype(np.float32)


def kernel(embeddings, labels, bank_embs, bank_labels, class_temps):
    global LAST_EXEC_TIME_NS
    import ml_dtypes

    emb = np.asarray(embeddings, dtype=np.float32)
    bank = np.asarray(bank_embs, dtype=np.float32)
    lab = np.asarray(labels).astype(np.int64).ravel()
    blab = np.asarray(bank_labels).astype(np.int64).ravel()
    ct = np.asarray(class_temps, dtype=np.float32).ravel()

    bord = np.argsort(lab, kind="stable")
    slab = lab[bord]
    mord = np.argsort(blab, kind="stable")
    cnt = np.bincount(lab, minlength=C)
    mcnt = np.bincount(blab, minlength=C)
    bb_b1, bb_b2 = int(cnt[0]), int(cnt[0] + cnt[1])
    mk_b1, mk_b2 = int(mcnt[0]), int(mcnt[0] + mcnt[1])

    embT = np.ascontiguousarray(emb[bord].T)      # [D, B]
    bankT = np.ascontiguousarray(bank[mord].T)    # [D, M]
    if MM_MODE == "bf16":
        embT = embT.astype(ml_dtypes.bfloat16)
        bankT = bankT.astype(ml_dtypes.bfloat16)

    temps = ct[slab]
    inv_t = (1.0 / temps).astype(np.float32)
    pos_cnt = cnt[slab] - 1
    invpc = (1.0 / np.maximum(pos_cnt, 1)).astype(np.float32)
    validf = (pos_cnt > 0).astype(np.float32)
    coefv = (BASE_TEMP / temps).astype(np.float32) * validf
    oneh = np.eye(C, dtype=np.float32)[slab]      # [B, 3]
    n_valid = int((pos_cnt > 0).sum())

    nc = _build(bb_b1, bb_b2, mk_b1, mk_b2, MM_MODE)

    subs, _, _ = _bank_subranges(mk_b1, mk_b2)
    NK = len(subs)
    sub_cls = np.array([0 if s < mk_b1 else (1 if s < mk_b2 else 2) for s, _ in subs])
    # incl[anchor, k] = 1 where subrange class != anchor class
    incl_full = (sub_cls[None, :] != slab[:, None]).astype(np.float32)  # [B, NK]
    eye128 = np.eye(128, dtype=np.float32)

    # per-class embedding-sum vectors for the positives matmul
    gT = np.stack([emb[bord][slab == c].sum(axis=0) for c in range(C)], axis=1)
    gT = np.ascontiguousarray(gT).astype(embT.dtype)

    in_maps = []
    for core in range(NCORES):
        asl = slice(core * APC, (core + 1) * APC)
        oh = oneh[asl].reshape(NT, 128, C).transpose(1, 0, 2).reshape(128, NT * C)
        ic = incl_full[asl].reshape(NT, 128, NK).transpose(1, 0, 2).reshape(128, NT * NK)
        vecs = np.concatenate([
            _per_core_cols(inv_t, core),
            _per_core_cols(-inv_t, core),
            _per_core_cols(invpc, core),
            _per_core_cols(coefv, core),
            oh.astype(np.float32),
            ic.astype(np.float32),
            eye128,
        ], axis=1)
        in_maps.append({
            "embT": embT,
            "anchT": np.ascontiguousarray(np.concatenate([embT[:, asl], gT], axis=1)),
            "bankT": bankT,
            "vecs": np.ascontiguousarray(vecs),
        })

    trace = os.environ.get("SUPCON_TRACE", "0") == "1"
    if trace:
        trace = _install_trace_shim()
    res = run_bass_kernel_spmd(nc, in_maps, core_ids=list(range(NCORES)), trace=trace)
    LAST_EXEC_TIME_NS = res.exec_time_ns

    # loss_i = coef_i * log(den_i) + lin_i ; device produced den/lin,
    # host finishes the 2048 scalar logs + masked mean
    loss_sum = np.float64(0.0)
    for core in range(NCORES):
        oo = np.asarray(res.results[core]["oout"], dtype=np.float64)    # [128, 2*NT]
        den, lin = oo[:, :NT], oo[:, NT:]
        cf = _per_core_cols(coefv, core).astype(np.float64)
        loss_sum += (cf * np.log(den) + lin).sum()
    return np.float32(loss_sum / max(n_valid, 1))



# revision 4
# speedup vs baseline: 1.0509x; 1.0509x over previous
"""ClassBalancedSupConLoss on 8 TRN2 NeuronCores (Bass/Tile), v2.

Math (reference semantics, reorganized for hardware):
  - All embeddings are unit-norm; fixed logsumexp shift m = 1:
        LSE_i = inv_t_i + log( sum_j exp(inv_t_i * (s_ij - 1)) )
    Self term excluded by subtracting exp of the bitwise-identical
    on-device s_ii product.  Batch and bank sorted by class on host so
    same-class columns are contiguous segments.
  - Anchors sharded 256/core across 8 cores; full embT/bankT replicas
    per core.  Device outputs per-anchor (den, lin); host does the
    2048 logs + masked mean.

v2 changes vs baseline (65.6us):
  - Input DMAs issued from sync+gpsimd queues only -- the scalar (ACT)
    engine previously spent ~9.5us issuing DMA descriptors before its
    first exp.
  - The exp stream is SPLIT between the ACT engine (hardware exp LUT,
    1 elem/lane/cyc @ 1.2GHz) and the Vector engine via two custom DVE
    ops: exp(z) ~= p3(z/128)^128 where p3 is a minimax cubic in
    factored form (pass A, 6 ALU slices) and pass B is 7 inline
    squarings with a free row-sum accumulator.  Max rel err 3.7e-4.
  - Anchor (stationary) operands are pre-scaled by inv_t/128 on host,
    so PSUM holds z/128 directly: ACT chunks use exp(128*x - inv_t)
    (free affine), DVE pass A needs only a per-partition shift.

SPMD: one program for all 8 cores; per-core data in the packed `vecs`
tile.
"""

import os
import numpy as np

import concourse.bass as bass  # noqa: F401
from concourse import bacc
import concourse.mybir as mybir
import concourse.tile as tile
from concourse.bass_utils import run_bass_kernel_spmd

B, D, M, C = 2048, 128, 16384, 3
NCORES = 8
APC = B // NCORES          # anchors per core = 256
NT = APC // 128            # anchor tiles per core = 2
CH = 512                   # matmul free chunk (one PSUM bank)
W = 2048                   # big PSUM chunk (4 banks) = one exp pass
NBK = M // W               # 8 bank pieces of [128, 2048]
BASE_TEMP = 0.07

F32 = mybir.dt.float32
BF16 = mybir.dt.bfloat16
AF = mybir.ActivationFunctionType
ALU = mybir.AluOpType
AX = mybir.AxisListType

LAST_EXEC_TIME_NS = None   # set by kernel() when SUPCON_TRACE=1

# ---- custom DVE exp: exp(z) = p3(v)^128, v = z/128 - u0 --------------------
# p3 fit of e^u on u in [-0.26, 0] (z in [-33, 0]; terms below e^-33 are
# ~1e-14 of the row sum).  Factored: p = (v*k) * ((v + bp)*v + gp),
# v = Src0 + C0 with C0 = -inv_t/128 - u0 per-anchor.  Pass B: w^128 via
# 7 squarings, row-sum accumulated in-instruction (no READ_ACCUMULATOR).
EXP_U0 = -1.7295465562795673
EXP_K = 0.146172629836262
EXP_BP = -1.791396476586659
EXP_GP = 4.062464246444453

_EXP_OPS = {}


def _register_exp_ops():
    """Define + register the two custom DVE ops with concourse's tables.

    dve_table_for_ops resolves op names through dve_ops.OPS /
    _SUB_OPCODE_FOR_NAME / CUSTOM_DVE_SPECS, all module-level registries;
    new ops just take the next free 5-bit opcode rows (18 in use < 32).
    """
    global _EXP_OPS
    if _EXP_OPS:
        return _EXP_OPS
    from operator import add as _add
    import concourse.dve_ops as dom
    from concourse.dve_spec import Spec, Src0, Src1, C0, C1, C2, lower, sq
    from concourse.dve_spec import _has_src1
    from concourse.dve_uop import DveOpSpec

    v = Src0 + C0
    body_a = (v * Src1) * ((v + C1) * v + C2)

    def ref_a(in0, in1, s0, s1, imm2):
        vv = in0.astype(np.float32) + np.asarray(s0, np.float32)
        return ((vv * np.asarray(in1, np.float32).reshape(-1, 1))
                * ((vv + s1) * vv + imm2)).astype(np.float32)

    body_b = sq(sq(sq(sq(sq(sq(sq(Src0)))))))

    def ref_b(in0, in1, s0, s1, imm2):
        b = in0.astype(np.float32)
        for _ in range(7):
            b = (b * b).astype(np.float32)
        acc = np.asarray(s0, np.float32).reshape(-1, 1) + b.reshape(
            b.shape[0], -1).sum(axis=-1, keepdims=True)
        return b, acc.astype(np.float32)

    spec_a = Spec(body=body_a, reference=ref_a)
    spec_b = Spec(body=body_b, accum=_add, accum_init=C0, reference=ref_b)

    for name, spec in [("EXP_POLY_A_ANT", spec_a), ("EXP_SQ7_RED_ANT", spec_b)]:
        if name in dom._SUB_OPCODE_FOR_NAME:
            _EXP_OPS[name] = next(o for o in dom.OPS if o.name == name)
            continue
        row = max(dom._SUB_OPCODE_FOR_NAME.values()) + 1
        assert row < 0x20
        dom._SUB_OPCODE_FOR_NAME[name] = row
        shas = {}
        for ver in ("v3", "v4"):
            try:
                r = DveOpSpec(name=name, opcode=row, uops=lower(spec, ver=ver),
                              rd1_en=_has_src1(spec))
                shas[ver] = r.sha(ver)
            except Exception:
                pass
        op = dom.DveOp(name, spec, subdim=False, uops_sha=shas)
        dom.OPS.append(op)
        dom.CUSTOM_DVE_SPECS[name] = spec
        _EXP_OPS[name] = op
    return _EXP_OPS


def _install_trace_shim():
    """Register the NTFF profile hook that this image's antenv lacks."""
    import sys
    import types
    import ctypes
    import contextlib

    try:
        from antenv.axon_hooks import get_axon_ntff_profile_hook  # noqa: F401
        return True
    except ImportError:
        pass

    so_path = "/opt/axon/libaxon_pjrt.so"
    if not os.path.exists(so_path):
        return False
    lib = ctypes.CDLL(so_path)
    if not hasattr(lib, "axon_start_nrt_profile"):
        return False
    lib.axon_start_nrt_profile.argtypes = [
        ctypes.POINTER(ctypes.c_int64),
        ctypes.c_size_t,
    ]
    lib.axon_start_nrt_profile.restype = ctypes.c_int64
    lib.axon_stop_nrt_profile.argtypes = [ctypes.c_char_p]
    lib.axon_stop_nrt_profile.restype = ctypes.c_int64

    @contextlib.contextmanager
    def _hook(output_dir, device_ids):
        import jax

        jax.devices()
        if device_ids:
            ids = (ctypes.c_int64 * len(device_ids))(*device_ids)
            rc = lib.axon_start_nrt_profile(ids, len(device_ids))
        else:
            rc = lib.axon_start_nrt_profile(None, 0)
        if rc != 0:
            raise RuntimeError(f"axon_start_nrt_profile rc={rc}")
        try:
            yield
        finally:
            n = lib.axon_stop_nrt_profile(str(output_dir).encode())
            print(f"profile: {n} file(s) written to {output_dir}", file=sys.stderr)

    _state = {"hook": _hook}
    mod = types.ModuleType("antenv.axon_hooks")
    mod.get_axon_ntff_profile_hook = lambda: _state["hook"]
    mod.set_axon_ntff_profile_hook = lambda h: _state.update(hook=h)
    sys.modules["antenv.axon_hooks"] = mod
    import antenv

    antenv.axon_hooks = mod

    import concourse.bass_utils as bu

    bu.upload_artifacts = lambda tmpdir: tmpdir
    return True


def _bank_subranges(mk_b1, mk_b2):
    """Split [0, M) at big-chunk multiples AND class boundaries."""
    cuts = sorted({c * W for c in range(NBK + 1)} | {mk_b1, mk_b2})
    subs = [(cuts[i], cuts[i + 1]) for i in range(len(cuts) - 1)]
    return subs


def _dve_chunks(mk_b1, mk_b2):
    """Bank chunk indices handled by the Vector engine (rest go to ACT).

    Boundary-containing chunks go to DVE (its per-call overhead is tiny,
    ACT pays ~600ns per extra call+accum-read), topped up for load
    balance: ACT chunk ~2.34us vs DVE dual-pass ~4.6us."""
    env = os.environ.get("SUPCON_DVE_CHUNKS")
    if env is not None:
        return {int(x) for x in env.split(",") if x != ""}
    picks = {mk_b1 // W, mk_b2 // W}
    for cand in (7, 3, 0, 6, 1, 4):
        if len(picks) >= 3:
            break
        picks.add(cand)
    return picks


def _build(mk_b1, mk_b2):
    import ml_dtypes  # noqa: F401

    ops = _register_exp_ops()
    EXP_A = ops["EXP_POLY_A_ANT"]
    EXP_B = ops["EXP_SQ7_RED_ANT"]

    nc = bacc.Bacc()
    # anchT = [scaled anchors (APC) | unscaled anchors (APC) | g_c (C)]
    embT_d = nc.declare_dram_parameter("embT", [D, B], BF16, isOutput=False)
    anchT_d = nc.declare_dram_parameter("anchT", [D, 2 * APC + C], BF16, isOutput=False)
    bankT_d = nc.declare_dram_parameter("bankT", [D, M], BF16, isOutput=False)
    subs = _bank_subranges(mk_b1, mk_b2)
    NK = len(subs)
    dvec = _dve_chunks(mk_b1, mk_b2)
    # vecs: [invt | ninvt | invpc | coefv | cA | oneh | incl | kcol | eye]
    NV = NT * (5 + C + NK) + 1 + 128
    vecs_d = nc.declare_dram_parameter("vecs", [128, NV], F32, isOutput=False)
    oout_d = nc.declare_dram_parameter("oout", [128, 2 * NT], F32, isOutput=True)

    with tile.TileContext(nc) as tc:
        with (
            tc.tile_pool(name="big", bufs=1) as bigp,
            tc.tile_pool(name="sm", bufs=1) as smp,
            tc.tile_pool(name="ps", bufs=2, space="PSUM") as psp,
        ):
            anch_t = bigp.tile([D, 2 * APC + C], BF16, tag="anchT")
            vecs_t = smp.tile([128, NV], F32, tag="vecs")
            junkw_t = bigp.tile([128, 128], BF16, tag="junkw")
            junkx_t = bigp.tile([128, CH], BF16, tag="junkx")
            o = [0]

            def vslice(w):
                a = o[0]; o[0] += w
                return vecs_t[:, a:a + w]

            invt_t = vslice(NT)
            ninvt_t = vslice(NT)
            invpc_t = vslice(NT)
            coefv_t = vslice(NT)
            cA_t = vslice(NT)
            oneh_t = vslice(NT * C)
            incl_t = vslice(NT * NK)
            kcol_t = vslice(1)
            eye_t = vslice(128)

            emb_t = bigp.tile([D, B], BF16, tag="embT")
            bank_ts = [bigp.tile([D, W], BF16, tag=f"bank{j}", name=f"bank{j}")
                       for j in range(NBK)]
            H = B // 2
            Q = B // 4
            # sync + gpsimd queues; scalar engine issues nothing
            eng2 = nc.gpsimd if os.environ.get("SUPCON_GPSIMD_DMA", "1") == "1" else nc.sync
            nc.sync.dma_start(out=vecs_t[:], in_=vecs_d[:])
            nc.sync.dma_start(out=anch_t[:], in_=anchT_d[:])
            eng2.dma_start(out=emb_t[:, 0:Q], in_=embT_d[:, 0:Q])
            nc.sync.dma_start(out=emb_t[:, Q:H], in_=embT_d[:, Q:H])
            eng2.dma_start(out=emb_t[:, H:H + Q], in_=embT_d[:, H:H + Q])
            nc.sync.dma_start(out=emb_t[:, H + Q:B], in_=embT_d[:, H + Q:B])
            eng2.dma_start(out=bank_ts[0][:, 0:H], in_=bankT_d[:, 0:H])
            nc.sync.dma_start(out=bank_ts[0][:, H:W], in_=bankT_d[:, H:W])
            eng2.dma_start(out=bank_ts[1][:, 0:H], in_=bankT_d[:, W:W + H])
            nc.sync.dma_start(out=bank_ts[1][:, H:W], in_=bankT_d[:, W + H:2 * W])
            for j in range(2, NBK):
                eng = nc.sync if j % 2 == 0 else eng2
                eng.dma_start(out=bank_ts[j][:], in_=bankT_d[:, j * W:(j + 1) * W])

            oout_t = smp.tile([128, 2 * NT], F32, tag="oout")
            scr_t = smp.tile([128, W], F32, tag="scrshared")
            wbuf_t = smp.tile([128, W], F32, tag="wbuf")
            dumb_t = smp.tile([128, W], BF16, tag="dumb")
            sdiag = [smp.tile([128, 1], F32, tag=f"sdiag{t}", name=f"sdiag{t}") for t in range(NT)]
            selfe = [smp.tile([128, 1], F32, tag=f"selfe{t}", name=f"selfe{t}") for t in range(NT)]
            eyemul = smp.tile([128, 128], F32, tag="eyemul")
            warm = smp.tile([128, 1], F32, tag="warm")
            bbsum = [smp.tile([128, 1], F32, tag=f"bbsum{t}", name=f"bbsum{t}") for t in range(NT)]
            raw3 = [smp.tile([128, C], F32, tag=f"raw3{t}", name=f"raw3{t}") for t in range(NT)]
            esum = [smp.tile([128, NK], F32, tag=f"esum{t}", name=f"esum{t}") for t in range(NT)]

            # pull the Exp table load off the critical path
            nc.scalar.activation(warm[:], eye_t[:, 0:1], AF.Exp)

            def anch(t):
                return anch_t[:, t * 128:(t + 1) * 128]

            def anchu(t):
                return anch_t[:, APC + t * 128:APC + (t + 1) * 128]

            # HAM warmup: ~4.3us of PE activity with garbage operands
            nc.vector.memset(junkw_t[:], 0.0)
            nc.vector.memset(junkx_t[:], 0.0)
            warm_ps = psp.tile([128, W], F32, tag="chunk", name="warm_ps")
            for w in range(8):
                nc.tensor.matmul(
                    warm_ps[:, (w % 4) * CH:((w % 4) + 1) * CH],
                    junkw_t[:], junkx_t[:], start=True, stop=True,
                )

            # ---- prelude: self blocks (scaled x unscaled) + positives ----
            pre_ps = psp.tile([128, W], F32, tag="chunk", name="pre_ps")
            for t in range(NT):
                nc.tensor.matmul(
                    pre_ps[:, t * 128:(t + 1) * 128], anch(t), anchu(t),
                    start=True, stop=True,
                )
            for t in range(NT):
                nc.tensor.matmul(
                    pre_ps[:, 256 + t * C:256 + (t + 1) * C], anch(t),
                    anch_t[:, 2 * APC:2 * APC + C], start=True, stop=True,
                )
            for t in range(NT):
                nc.vector.tensor_mul(eyemul[:], pre_ps[:, t * 128:(t + 1) * 128], eye_t[:])
                nc.vector.reduce_sum(sdiag[t][:], eyemul[:], axis=AX.X)
                nc.vector.tensor_copy(out=raw3[t][:], in_=pre_ps[:, 256 + t * C:256 + (t + 1) * C])
                nc.scalar.activation(
                    selfe[t][:], sdiag[t][:], AF.Exp,
                    bias=ninvt_t[:, t:t + 1], scale=128.0,
                )

            by_chunk = {}
            for k, (s, e) in enumerate(subs):
                by_chunk.setdefault(s // W, []).append((s, e, k))

            scrNK = [smp.tile([128, NK], F32, tag=f"scrNK{t}", name=f"scrNK{t}") for t in range(NT)]
            scrC = [smp.tile([128, C], F32, tag=f"scrC{t}", name=f"scrC{t}") for t in range(NT)]

            def epi_early(t):
                """olin = coefv*invt*(1 - pos): prelude-only deps."""
                own_r = smp.tile([128, 1], F32, tag=f"ownr{t}", name=f"ownr{t}")
                pos = smp.tile([128, 1], F32, tag=f"pos{t}", name=f"pos{t}")
                w1 = smp.tile([128, 1], F32, tag=f"w1{t}", name=f"w1{t}")
                p1 = smp.tile([128, 1], F32, tag=f"p1{t}", name=f"p1{t}")
                nc.vector.tensor_mul(scrC[t][:], raw3[t][:], oneh_t[:, t * C:(t + 1) * C])
                nc.vector.reduce_sum(own_r[:], scrC[t][:], axis=AX.X)
                nc.vector.scalar_tensor_tensor(
                    out=pos[:], in0=own_r[:], scalar=sdiag[t][:], in1=invpc_t[:, t:t + 1],
                    op0=ALU.subtract, op1=ALU.mult,
                )
                nc.vector.scalar_tensor_tensor(
                    out=w1[:], in0=pos[:], scalar=-1.0, in1=invt_t[:, t:t + 1],
                    op0=ALU.mult, op1=ALU.mult,
                )
                nc.vector.scalar_tensor_tensor(
                    out=oout_t[:, NT + t:NT + t + 1], in0=w1[:], scalar=invt_t[:, t:t + 1],
                    in1=coefv_t[:, t:t + 1], op0=ALU.add, op1=ALU.mult,
                )
                return p1

            p1s = {}

            def epilogue(t):
                nc.vector.tensor_mul(scrNK[t][:], esum[t][:], incl_t[:, t * NK:(t + 1) * NK])
                nc.vector.reduce_sum(oout_t[:, t:t + 1], scrNK[t][:], axis=AX.X)
                nc.vector.tensor_add(oout_t[:, t:t + 1], oout_t[:, t:t + 1], p1s[t][:])

            def emit_bb(t):
                ps = psp.tile([128, W], F32, tag="chunk", name="bb_ps")
                for q in range(W // CH):
                    nc.tensor.matmul(
                        ps[:, q * CH:(q + 1) * CH], anch(t),
                        emb_t[:, q * CH:(q + 1) * CH],
                        start=True, stop=True,
                    )
                nc.scalar.activation(
                    scr_t[:], ps[:], AF.Exp,
                    bias=ninvt_t[:, t:t + 1], scale=128.0,
                    accum_out=bbsum[t][:],
                )

            def emit_bank_act(t, j):
                ps = psp.tile([128, W], F32, tag="chunk", name="bk_ps")
                for q in range(W // CH):
                    nc.tensor.matmul(
                        ps[:, q * CH:(q + 1) * CH], anch(t),
                        bank_ts[j][:, q * CH:(q + 1) * CH],
                        start=True, stop=True,
                    )
                for (s, e, k) in by_chunk[j]:
                    a, b = s - j * W, e - j * W
                    nc.scalar.activation(
                        scr_t[:, a:b], ps[:, a:b], AF.Exp,
                        bias=ninvt_t[:, t:t + 1], scale=128.0,
                        accum_out=esum[t][:, k:k + 1],
                    )

            def emit_bank_dve(t, j):
                ps = psp.tile([128, W], F32, tag="chunk", name="dk_ps")
                for q in range(W // CH):
                    nc.tensor.matmul(
                        ps[:, q * CH:(q + 1) * CH], anch(t),
                        bank_ts[j][:, q * CH:(q + 1) * CH],
                        start=True, stop=True,
                    )
                nc.vector._custom_dve(
                    EXP_A, out=wbuf_t[:], in0=ps[:], in1=kcol_t[:],
                    s0=cA_t[:, t:t + 1], s1=EXP_BP, imm2=EXP_GP,
                )
                for (s, e, k) in by_chunk[j]:
                    a, b = s - j * W, e - j * W
                    nc.vector._custom_dve(
                        EXP_B, out=dumb_t[:, a:b], in0=wbuf_t[:, a:b],
                        s0=0.0, accum_out=esum[t][:, k:k + 1],
                    )

            def emit_bank(t, j):
                if j in dvec:
                    emit_bank_dve(t, j)
                else:
                    emit_bank_act(t, j)

            for t in range(NT):
                p1s[t] = epi_early(t)
            emit_bb(0)
            nc.vector.tensor_sub(p1s[0][:], bbsum[0][:], selfe[0][:])
            for j in range(NBK - 1):
                emit_bank(0, j)
            emit_bb(1)
            nc.vector.tensor_sub(p1s[1][:], bbsum[1][:], selfe[1][:])
            emit_bank(0, NBK - 1)
            epilogue(0)
            for j in range(NBK):
                emit_bank(1, j)
            epilogue(1)

            nc.sync.dma_start(out=oout_d[:], in_=oout_t[:])

    nc.compile()
    return nc


def _per_core_cols(vec, core):
    """[B] host vector -> [128, NT] tile for one core (col t, partition p)."""
    sl = vec[core * APC:(core + 1) * APC]
    return np.ascontiguousarray(sl.reshape(NT, 128).T).astype(np.float32)


def kernel(embeddings, labels, bank_embs, bank_labels, class_temps):
    global LAST_EXEC_TIME_NS
    import ml_dtypes

    emb = np.asarray(embeddings, dtype=np.float32)
    bank = np.asarray(bank_embs, dtype=np.float32)
    lab = np.asarray(labels).astype(np.int64).ravel()
    blab = np.asarray(bank_labels).astype(np.int64).ravel()
    ct = np.asarray(class_temps, dtype=np.float32).ravel()

    bord = np.argsort(lab, kind="stable")
    slab = lab[bord]
    mord = np.argsort(blab, kind="stable")
    cnt = np.bincount(lab, minlength=C)
    mcnt = np.bincount(blab, minlength=C)
    mk_b1, mk_b2 = int(mcnt[0]), int(mcnt[0] + mcnt[1])

    embT = np.ascontiguousarray(emb[bord].T).astype(ml_dtypes.bfloat16)  # [D, B]
    bankT = np.ascontiguousarray(bank[mord].T).astype(ml_dtypes.bfloat16)  # [D, M]

    temps = ct[slab]
    inv_t = (1.0 / temps).astype(np.float32)
    pos_cnt = cnt[slab] - 1
    # positives matmul is scaled by inv_t/128 (pre-scaled anchors)
    invpc = (128.0 / inv_t / np.maximum(pos_cnt, 1)).astype(np.float32)
    validf = (pos_cnt > 0).astype(np.float32)
    coefv = (BASE_TEMP / temps).astype(np.float32) * validf
    oneh = np.eye(C, dtype=np.float32)[slab]      # [B, 3]
    n_valid = int((pos_cnt > 0).sum())

    nc = _build(mk_b1, mk_b2)

    subs = _bank_subranges(mk_b1, mk_b2)
    NK = len(subs)
    sub_cls = np.array([0 if s < mk_b1 else (1 if s < mk_b2 else 2) for s, _ in subs])
    incl_full = (sub_cls[None, :] != slab[:, None]).astype(np.float32)  # [B, NK]
    eye128 = np.eye(128, dtype=np.float32)

    # per-class embedding-sum vectors for the positives matmul (unscaled)
    gT = np.stack([emb[bord][slab == c].sum(axis=0) for c in range(C)], axis=1)
    gT = np.ascontiguousarray(gT).astype(ml_dtypes.bfloat16)

    # DVE pass A per-anchor shift: C0 = -inv_t/128 - u0
    cA = (-inv_t / 128.0 - EXP_U0).astype(np.float32)
    kcol = np.full((128, 1), EXP_K, dtype=np.float32)

    in_maps = []
    for core in range(NCORES):
        asl = slice(core * APC, (core + 1) * APC)
        oh = oneh[asl].reshape(NT, 128, C).transpose(1, 0, 2).reshape(128, NT * C)
        ic = incl_full[asl].reshape(NT, 128, NK).transpose(1, 0, 2).reshape(128, NT * NK)
        vecs = np.concatenate([
            _per_core_cols(inv_t, core),
            _per_core_cols(-inv_t, core),
            _per_core_cols(invpc, core),
            _per_core_cols(coefv, core),
            _per_core_cols(cA, core),
            oh.astype(np.float32),
            ic.astype(np.float32),
            kcol,
            eye128,
        ], axis=1)
        # scaled anchors: columns * inv_t_i/128 (scale BEFORE bf16 cast)
        anch_sc = (emb[bord][asl] * (inv_t[asl, None] / 128.0)).T.astype(ml_dtypes.bfloat16)
        anch_un = embT[:, asl]
        in_maps.append({
            "embT": embT,
            "anchT": np.ascontiguousarray(
                np.concatenate([anch_sc, anch_un, gT], axis=1)),
            "bankT": bankT,
            "vecs": np.ascontiguousarray(vecs),
        })

    trace = os.environ.get("SUPCON_TRACE", "0") == "1"
    if trace:
        trace = _install_trace_shim()
    res = run_bass_kernel_spmd(nc, in_maps, core_ids=list(range(NCORES)), trace=trace)
    LAST_EXEC_TIME_NS = res.exec_time_ns

    # loss_i = coef_i * log(den_i) + lin_i; host finishes logs + masked mean
    loss_sum = np.float64(0.0)
    for core in range(NCORES):
        oo = np.asarray(res.results[core]["oout"], dtype=np.float64)    # [128, 2*NT]
        den, lin = oo[:, :NT], oo[:, NT:]
        cf = _per_core_cols(coefv, core).astype(np.float64)
        loss_sum += (cf * np.log(den) + lin).sum()
    return np.float32(loss_sum / max(n_valid, 1))


# revision 14
# speedup vs baseline: 1.1080x; 1.0543x over previous
"""ClassBalancedSupConLoss on 8 TRN2 NeuronCores (Bass/Tile), v2.

Math (reference semantics, reorganized for hardware):
  - All embeddings are unit-norm; fixed logsumexp shift m = 1:
        LSE_i = inv_t_i + log( sum_j exp(inv_t_i * (s_ij - 1)) )
    Self term excluded by subtracting exp of the bitwise-identical
    on-device s_ii product.  Batch and bank sorted by class on host so
    same-class columns are contiguous segments.
  - Anchors sharded 256/core across 8 cores; full embT/bankT replicas
    per core.  Device outputs per-anchor (den, lin); host does the
    2048 logs + masked mean.

v2 changes vs baseline (65.6us):
  - Input DMAs issued from sync+gpsimd queues only -- the scalar (ACT)
    engine previously spent ~9.5us issuing DMA descriptors before its
    first exp.
  - The exp stream is SPLIT between the ACT engine (hardware exp LUT,
    1 elem/lane/cyc @ 1.2GHz) and the Vector engine via two custom DVE
    ops: exp(z) ~= p3(z/128)^128 where p3 is a minimax cubic in
    factored form (pass A, 6 ALU slices) and pass B is 7 inline
    squarings with a free row-sum accumulator.  Max rel err 3.7e-4.
  - Anchor (stationary) operands are pre-scaled by inv_t/128 on host,
    so PSUM holds z/128 directly: ACT chunks use exp(128*x - inv_t)
    (free affine), DVE pass A needs only a per-partition shift.

SPMD: one program for all 8 cores; per-core data in the packed `vecs`
tile.
"""

import os
import numpy as np

import concourse.bass as bass  # noqa: F401
from concourse import bacc
import concourse.mybir as mybir
import concourse.tile as tile
from concourse.bass_utils import run_bass_kernel_spmd

B, D, M, C = 2048, 128, 16384, 3
NCORES = 8
APC = B // NCORES          # anchors per core = 256
NT = APC // 128            # anchor tiles per core = 2
CH = 512                   # matmul free chunk (one PSUM bank)
W = 2048                   # big PSUM chunk (4 banks) = one exp pass
NBK = M // W               # 8 bank pieces of [128, 2048]
BASE_TEMP = 0.07

F32 = mybir.dt.float32
BF16 = mybir.dt.bfloat16
AF = mybir.ActivationFunctionType
ALU = mybir.AluOpType
AX = mybir.AxisListType

LAST_EXEC_TIME_NS = None   # set by kernel() when SUPCON_TRACE=1

# ---- custom DVE exp: exp(z) = p3(v)^128, v = z/128 - u0 --------------------
# p3 fit of e^u on u in [-0.26, 0] (z in [-33, 0]; terms below e^-33 are
# ~1e-14 of the row sum).  Factored: p = (v*k) * ((v + bp)*v + gp),
# v = Src0 + C0 with C0 = -inv_t/128 - u0 per-anchor.  Pass B: w^128 via
# 7 squarings, row-sum accumulated in-instruction (no READ_ACCUMULATOR).
EXP_U0 = -1.7295465562795673
EXP_K = 0.146172629836262
EXP_BP = -1.791396476586659
EXP_GP = 4.062464246444453
# k absorbed into the variable: w = cbrt(k)*v, p = w*((w+BPk)*w+GPk);
# host scales anchors by inv_t/128*cbrt(k), ACT uses scale 128/cbrt(k)
KCBRT = float(EXP_K ** (1.0 / 3.0))
EXP_BPK = float(EXP_BP * KCBRT)
EXP_GPK = float(EXP_GP * KCBRT * KCBRT)
ACT_SCALE = float(128.0 / KCBRT)

_EXP_OPS = {}


def _register_exp_ops():
    """Define + register the two custom DVE ops with concourse's tables.

    dve_table_for_ops resolves op names through dve_ops.OPS /
    _SUB_OPCODE_FOR_NAME / CUSTOM_DVE_SPECS, all module-level registries;
    new ops just take the next free 5-bit opcode rows (18 in use < 32).
    """
    global _EXP_OPS
    if _EXP_OPS:
        return _EXP_OPS
    from operator import add as _add
    import concourse.dve_ops as dom
    from concourse.dve_spec import Spec, Src0, Src1, C0, C1, C2, lower, sq
    from concourse.dve_spec import _has_src1
    from concourse.dve_uop import DveOpSpec

    # pass A: q = v*((v+bp)*v+gp), v = Src0 + C0; the leading k is applied
    # in pass B's first slice (q*C1 before the squarings) -- keeps pass A
    # at three constant slots with no Src1 stream.
    tmode = os.environ.get("SUPCON_DVE_SPEC", "fan3")
    if tmode == "fan2":
        va = Src0 + C0
        vb = Src0 + C0
        body_a = ((va + C1) * va + C2) * vb
    else:
        v = Src0 + C0
        body_a = v * ((v + C1) * v + C2)

    def ref_a(in0, in1, s0, s1, imm2):
        vv = in0.astype(np.float32) + np.asarray(s0, np.float32)
        return (vv * ((vv + s1) * vv + imm2)).astype(np.float32)

    body_b = sq(sq(sq(sq(sq(sq(sq(Src0)))))))

    def ref_b(in0, in1, s0, s1, imm2):
        b = in0.astype(np.float32)
        for _ in range(7):
            b = (b * b).astype(np.float32)
        acc = np.asarray(s0, np.float32).reshape(-1, 1) + b.reshape(
            b.shape[0], -1).sum(axis=-1, keepdims=True)
        return b, acc.astype(np.float32)

    spec_a = Spec(body=body_a, reference=ref_a)
    spec_b = Spec(body=body_b, accum=_add, accum_init=C0, reference=ref_b)

    # The DVE NX firmware dispatch table only knows the stock opcode rows,
    # so new rows would hang the engine.  Instead REPLACE the table
    # programs of two stock ops this kernel never calls -- the per-NEFF
    # uop table (qDveTable) is regenerated from dve_ops.OPS at compile
    # time, so the hijacked rows carry the exp programs.
    hijack = {"EXP_POLY_A_ANT": "CODY_WAITE_CASCADE",
              "EXP_SQ7_RED_ANT": "ADD_RANGE_WRAP"}
    for myname, spec in [("EXP_POLY_A_ANT", spec_a), ("EXP_SQ7_RED_ANT", spec_b)]:
        name = hijack[myname]
        idx = next(i for i, o in enumerate(dom.OPS) if o.name == name)
        if dom.OPS[idx].spec is spec:
            _EXP_OPS[myname] = dom.OPS[idx]
            continue
        row = dom._SUB_OPCODE_FOR_NAME[name]
        shas = {}
        for ver in ("v3", "v4"):
            try:
                r = DveOpSpec(name=name, opcode=row, uops=lower(spec, ver=ver),
                              rd1_en=_has_src1(spec))
                shas[ver] = r.sha(ver)
            except Exception:
                pass
        op = dom.DveOp(name, spec, subdim=False, uops_sha=shas)
        dom.OPS[idx] = op
        dom.CUSTOM_DVE_SPECS[name] = spec
        _EXP_OPS[myname] = op
    return _EXP_OPS


def _install_trace_shim():
    """Register the NTFF profile hook that this image's antenv lacks."""
    import sys
    import types
    import ctypes
    import contextlib

    try:
        from antenv.axon_hooks import get_axon_ntff_profile_hook  # noqa: F401
        return True
    except ImportError:
        pass

    so_path = "/opt/axon/libaxon_pjrt.so"
    if not os.path.exists(so_path):
        return False
    lib = ctypes.CDLL(so_path)
    if not hasattr(lib, "axon_start_nrt_profile"):
        return False
    lib.axon_start_nrt_profile.argtypes = [
        ctypes.POINTER(ctypes.c_int64),
        ctypes.c_size_t,
    ]
    lib.axon_start_nrt_profile.restype = ctypes.c_int64
    lib.axon_stop_nrt_profile.argtypes = [ctypes.c_char_p]
    lib.axon_stop_nrt_profile.restype = ctypes.c_int64

    @contextlib.contextmanager
    def _hook(output_dir, device_ids):
        import jax

        jax.devices()
        if device_ids:
            ids = (ctypes.c_int64 * len(device_ids))(*device_ids)
            rc = lib.axon_start_nrt_profile(ids, len(device_ids))
        else:
            rc = lib.axon_start_nrt_profile(None, 0)
        if rc != 0:
            raise RuntimeError(f"axon_start_nrt_profile rc={rc}")
        try:
            yield
        finally:
            n = lib.axon_stop_nrt_profile(str(output_dir).encode())
            print(f"profile: {n} file(s) written to {output_dir}", file=sys.stderr)

    _state = {"hook": _hook}
    mod = types.ModuleType("antenv.axon_hooks")
    mod.get_axon_ntff_profile_hook = lambda: _state["hook"]
    mod.set_axon_ntff_profile_hook = lambda h: _state.update(hook=h)
    sys.modules["antenv.axon_hooks"] = mod
    import antenv

    antenv.axon_hooks = mod

    import concourse.bass_utils as bu

    bu.upload_artifacts = lambda tmpdir: tmpdir
    return True


def _bank_subranges(mk_b1, mk_b2):
    """Split [0, M) at big-chunk multiples AND class boundaries."""
    cuts = sorted({c * W for c in range(NBK + 1)} | {mk_b1, mk_b2})
    subs = [(cuts[i], cuts[i + 1]) for i in range(len(cuts) - 1)]
    return subs


def _dve_chunks(mk_b1, mk_b2):
    """Bank chunk indices handled by the Vector engine (rest go to ACT).

    Boundary-containing chunks go to DVE (its per-call overhead is tiny,
    ACT pays ~600ns per extra call+accum-read), topped up for load
    balance: ACT chunk ~2.34us vs DVE dual-pass ~4.6us."""
    env = os.environ.get("SUPCON_DVE_CHUNKS")
    if env is not None:
        return {int(x) for x in env.split(",") if x != ""}
    picks = {mk_b1 // W, mk_b2 // W}
    for cand in (7, 3, 0, 6, 1, 4):
        if len(picks) >= 3:
            break
        picks.add(cand)
    return picks


def _build(mk_b1, mk_b2):
    import ml_dtypes  # noqa: F401

    ops = _register_exp_ops()
    EXP_A = ops["EXP_POLY_A_ANT"]
    EXP_B = ops["EXP_SQ7_RED_ANT"]

    nc = bacc.Bacc()
    # anchT = [scaled anchors (APC) | unscaled anchors (APC) | g_c (C)]
    embT_d = nc.declare_dram_parameter("embT", [D, B], BF16, isOutput=False)
    anchT_d = nc.declare_dram_parameter("anchT", [D, 2 * APC + C], BF16, isOutput=False)
    bankT_d = nc.declare_dram_parameter("bankT", [D, M], BF16, isOutput=False)
    subs = _bank_subranges(mk_b1, mk_b2)
    NK = len(subs)
    dvec = _dve_chunks(mk_b1, mk_b2)
    # vecs: [invt | ninvt | invpc | coefv | cA | oneh | incl | kcol | eye]
    NV = NT * (5 + C + NK) + 1 + 128
    vecs_d = nc.declare_dram_parameter("vecs", [128, NV], F32, isOutput=False)
    oout_d = nc.declare_dram_parameter("oout", [128, 2 * NT], F32, isOutput=True)

    with tile.TileContext(nc) as tc:
        with (
            tc.tile_pool(name="big", bufs=1) as bigp,
            tc.tile_pool(name="sm", bufs=1) as smp,
            tc.tile_pool(name="ps", bufs=2, space="PSUM") as psp,
        ):
            anch_t = bigp.tile([D, 2 * APC + C], BF16, tag="anchT")
            vecs_t = smp.tile([128, NV], F32, tag="vecs")
            junkw_t = bigp.tile([128, 128], BF16, tag="junkw")
            junkx_t = bigp.tile([128, CH], BF16, tag="junkx")
            o = [0]

            def vslice(w):
                a = o[0]; o[0] += w
                return vecs_t[:, a:a + w]

            invt_t = vslice(NT)
            ninvt_t = vslice(NT)
            invpc_t = vslice(NT)
            coefv_t = vslice(NT)
            cA_t = vslice(NT)
            oneh_t = vslice(NT * C)
            incl_t = vslice(NT * NK)
            kcol_t = vslice(1)
            eye_t = vslice(128)

            emb_t = bigp.tile([D, B], BF16, tag="embT")
            bank_ts = [bigp.tile([D, W], BF16, tag=f"bank{j}", name=f"bank{j}")
                       for j in range(NBK)]
            H = B // 2
            Q = B // 4
            # sync + gpsimd queues; scalar engine issues nothing
            eng2 = nc.gpsimd if os.environ.get("SUPCON_GPSIMD_DMA", "0") == "1" else nc.sync
            nc.sync.dma_start(out=vecs_t[:], in_=vecs_d[:])
            nc.sync.dma_start(out=anch_t[:], in_=anchT_d[:])
            eng2.dma_start(out=emb_t[:, 0:Q], in_=embT_d[:, 0:Q])
            nc.sync.dma_start(out=emb_t[:, Q:H], in_=embT_d[:, Q:H])
            eng2.dma_start(out=emb_t[:, H:H + Q], in_=embT_d[:, H:H + Q])
            nc.sync.dma_start(out=emb_t[:, H + Q:B], in_=embT_d[:, H + Q:B])
            eng2.dma_start(out=bank_ts[0][:, 0:H], in_=bankT_d[:, 0:H])
            nc.sync.dma_start(out=bank_ts[0][:, H:W], in_=bankT_d[:, H:W])
            eng2.dma_start(out=bank_ts[1][:, 0:H], in_=bankT_d[:, W:W + H])
            nc.sync.dma_start(out=bank_ts[1][:, H:W], in_=bankT_d[:, W + H:2 * W])
            for j in range(2, NBK):
                eng = nc.sync if j % 2 == 0 else eng2
                eng.dma_start(out=bank_ts[j][:], in_=bankT_d[:, j * W:(j + 1) * W])

            oout_t = smp.tile([128, 2 * NT], F32, tag="oout")
            scr_t = smp.tile([128, W], F32, tag="scrshared")
            wbuf_t = smp.tile([128, W], F32, tag="wbuf")
            dumb_t = smp.tile([128, W], F32, tag="dumb")
            sdiag = [smp.tile([128, 1], F32, tag=f"sdiag{t}", name=f"sdiag{t}") for t in range(NT)]
            selfe = [smp.tile([128, 1], F32, tag=f"selfe{t}", name=f"selfe{t}") for t in range(NT)]
            eyemul = smp.tile([128, 128], F32, tag="eyemul")
            warm = smp.tile([128, 1], F32, tag="warm")
            bbsum = [smp.tile([128, 1], F32, tag=f"bbsum{t}", name=f"bbsum{t}") for t in range(NT)]
            raw3 = [smp.tile([128, C], F32, tag=f"raw3{t}", name=f"raw3{t}") for t in range(NT)]
            esum = [smp.tile([128, NK], F32, tag=f"esum{t}", name=f"esum{t}") for t in range(NT)]

            # pull the Exp table load off the critical path
            nc.scalar.activation(warm[:], eye_t[:, 0:1], AF.Exp)

            def anch(t):
                return anch_t[:, t * 128:(t + 1) * 128]

            def anchu(t):
                return anch_t[:, APC + t * 128:APC + (t + 1) * 128]

            # HAM warmup: ~4.3us of PE activity with garbage operands
            nc.vector.memset(junkw_t[:], 0.0)
            nc.vector.memset(junkx_t[:], 0.0)
            warm_ps = psp.tile([128, W], F32, tag="chunk", name="warm_ps")
            for w in range(8):
                nc.tensor.matmul(
                    warm_ps[:, (w % 4) * CH:((w % 4) + 1) * CH],
                    junkw_t[:], junkx_t[:], start=True, stop=True,
                )

            # ---- prelude: self blocks (scaled x unscaled) + positives ----
            pre_ps = psp.tile([128, W], F32, tag="chunk", name="pre_ps")
            for t in range(NT):
                nc.tensor.matmul(
                    pre_ps[:, t * 128:(t + 1) * 128], anch(t), anchu(t),
                    start=True, stop=True,
                )
            for t in range(NT):
                nc.tensor.matmul(
                    pre_ps[:, 256 + t * C:256 + (t + 1) * C], anch(t),
                    anch_t[:, 2 * APC:2 * APC + C], start=True, stop=True,
                )
            for t in range(NT):
                nc.vector.tensor_mul(eyemul[:], pre_ps[:, t * 128:(t + 1) * 128], eye_t[:])
                nc.vector.reduce_sum(sdiag[t][:], eyemul[:], axis=AX.X)
                nc.vector.tensor_copy(out=raw3[t][:], in_=pre_ps[:, 256 + t * C:256 + (t + 1) * C])
                nc.scalar.activation(
                    selfe[t][:], sdiag[t][:], AF.Exp,
                    bias=ninvt_t[:, t:t + 1], scale=ACT_SCALE,
                )

            by_chunk = {}
            for k, (s, e) in enumerate(subs):
                by_chunk.setdefault(s // W, []).append((s, e, k))

            scrNK = [smp.tile([128, NK], F32, tag=f"scrNK{t}", name=f"scrNK{t}") for t in range(NT)]
            scrC = [smp.tile([128, C], F32, tag=f"scrC{t}", name=f"scrC{t}") for t in range(NT)]

            def epi_early(t):
                """olin = coefv*invt*(1 - pos): prelude-only deps."""
                own_r = smp.tile([128, 1], F32, tag=f"ownr{t}", name=f"ownr{t}")
                pos = smp.tile([128, 1], F32, tag=f"pos{t}", name=f"pos{t}")
                w1 = smp.tile([128, 1], F32, tag=f"w1{t}", name=f"w1{t}")
                p1 = smp.tile([128, 1], F32, tag=f"p1{t}", name=f"p1{t}")
                nc.vector.tensor_mul(scrC[t][:], raw3[t][:], oneh_t[:, t * C:(t + 1) * C])
                nc.vector.reduce_sum(own_r[:], scrC[t][:], axis=AX.X)
                nc.vector.scalar_tensor_tensor(
                    out=pos[:], in0=own_r[:], scalar=sdiag[t][:], in1=invpc_t[:, t:t + 1],
                    op0=ALU.subtract, op1=ALU.mult,
                )
                nc.vector.scalar_tensor_tensor(
                    out=w1[:], in0=pos[:], scalar=-1.0, in1=invt_t[:, t:t + 1],
                    op0=ALU.mult, op1=ALU.mult,
                )
                nc.vector.scalar_tensor_tensor(
                    out=oout_t[:, NT + t:NT + t + 1], in0=w1[:], scalar=invt_t[:, t:t + 1],
                    in1=coefv_t[:, t:t + 1], op0=ALU.add, op1=ALU.mult,
                )
                return p1

            p1s = {}

            def epilogue(t):
                nc.vector.tensor_mul(scrNK[t][:], esum[t][:], incl_t[:, t * NK:(t + 1) * NK])
                nc.vector.reduce_sum(oout_t[:, t:t + 1], scrNK[t][:], axis=AX.X)
                nc.vector.tensor_add(oout_t[:, t:t + 1], oout_t[:, t:t + 1], p1s[t][:])

            def emit_bb(t):
                ps = psp.tile([128, W], F32, tag="chunk", name="bb_ps")
                for q in range(W // CH):
                    nc.tensor.matmul(
                        ps[:, q * CH:(q + 1) * CH], anch(t),
                        emb_t[:, q * CH:(q + 1) * CH],
                        start=True, stop=True,
                    )
                nc.scalar.activation(
                    scr_t[:], ps[:], AF.Exp,
                    bias=ninvt_t[:, t:t + 1], scale=ACT_SCALE,
                    accum_out=bbsum[t][:],
                )

            def emit_bank_act(t, j):
                ps = psp.tile([128, W], F32, tag="chunk", name="bk_ps")
                for q in range(W // CH):
                    nc.tensor.matmul(
                        ps[:, q * CH:(q + 1) * CH], anch(t),
                        bank_ts[j][:, q * CH:(q + 1) * CH],
                        start=True, stop=True,
                    )
                for (s, e, k) in by_chunk[j]:
                    a, b = s - j * W, e - j * W
                    nc.scalar.activation(
                        scr_t[:, a:b], ps[:, a:b], AF.Exp,
                        bias=ninvt_t[:, t:t + 1], scale=ACT_SCALE,
                        accum_out=esum[t][:, k:k + 1],
                    )

            dve_test = os.environ.get("SUPCON_DVE_TEST", "")

            def emit_bank_dve(t, j):
                ps = psp.tile([128, W], F32, tag="chunk", name="dk_ps")
                for q in range(W // CH):
                    nc.tensor.matmul(
                        ps[:, q * CH:(q + 1) * CH], anch(t),
                        bank_ts[j][:, q * CH:(q + 1) * CH],
                        start=True, stop=True,
                    )
                if dve_test == "stock":
                    # stock custom-dve op in place of pass A: wrong math,
                    # tests whether ANY custom-dve runs on this device
                    nc.vector.reciprocal_approx_fast(out=wbuf_t[:], in_=ps[:])
                else:
                    nc.vector._custom_dve(
                        EXP_A, out=wbuf_t[:], in0=ps[:],
                        s0=cA_t[:, t:t + 1], s1=EXP_BPK, imm2=EXP_GPK,
                    )
                for (s, e, k) in by_chunk[j]:
                    a, b = s - j * W, e - j * W
                    if dve_test in ("stock", "a"):
                        nc.vector.memset(esum[t][:, k:k + 1], 1.0)
                    else:
                        nc.vector._custom_dve(
                            EXP_B, out=dumb_t[:, a:b], in0=wbuf_t[:, a:b],
                            s0=0.0, accum_out=esum[t][:, k:k + 1],
                        )

            def emit_bank(t, j):
                if j in dvec:
                    emit_bank_dve(t, j)
                else:
                    emit_bank_act(t, j)

            for t in range(NT):
                p1s[t] = epi_early(t)
            emit_bb(0)
            nc.vector.tensor_sub(p1s[0][:], bbsum[0][:], selfe[0][:])
            for j in range(NBK - 1):
                emit_bank(0, j)
            emit_bb(1)
            nc.vector.tensor_sub(p1s[1][:], bbsum[1][:], selfe[1][:])
            emit_bank(0, NBK - 1)
            epilogue(0)
            for j in range(NBK):
                emit_bank(1, j)
            epilogue(1)

            nc.sync.dma_start(out=oout_d[:], in_=oout_t[:])

    nc.compile()
    return nc


def _per_core_cols(vec, core):
    """[B] host vector -> [128, NT] tile for one core (col t, partition p)."""
    sl = vec[core * APC:(core + 1) * APC]
    return np.ascontiguousarray(sl.reshape(NT, 128).T).astype(np.float32)


def kernel(embeddings, labels, bank_embs, bank_labels, class_temps):
    global LAST_EXEC_TIME_NS
    import ml_dtypes

    emb = np.asarray(embeddings, dtype=np.float32)
    bank = np.asarray(bank_embs, dtype=np.float32)
    lab = np.asarray(labels).astype(np.int64).ravel()
    blab = np.asarray(bank_labels).astype(np.int64).ravel()
    ct = np.asarray(class_temps, dtype=np.float32).ravel()

    bord = np.argsort(lab, kind="stable")
    slab = lab[bord]
    mord = np.argsort(blab, kind="stable")
    cnt = np.bincount(lab, minlength=C)
    mcnt = np.bincount(blab, minlength=C)
    mk_b1, mk_b2 = int(mcnt[0]), int(mcnt[0] + mcnt[1])

    embT = np.ascontiguousarray(emb[bord].T).astype(ml_dtypes.bfloat16)  # [D, B]
    bankT = np.ascontiguousarray(bank[mord].T).astype(ml_dtypes.bfloat16)  # [D, M]

    temps = ct[slab]
    inv_t = (1.0 / temps).astype(np.float32)
    pos_cnt = cnt[slab] - 1
    # positives matmul is scaled by inv_t/128 (pre-scaled anchors)
    invpc = (128.0 / KCBRT / inv_t / np.maximum(pos_cnt, 1)).astype(np.float32)
    validf = (pos_cnt > 0).astype(np.float32)
    coefv = (BASE_TEMP / temps).astype(np.float32) * validf
    oneh = np.eye(C, dtype=np.float32)[slab]      # [B, 3]
    n_valid = int((pos_cnt > 0).sum())

    nc = _build(mk_b1, mk_b2)

    subs = _bank_subranges(mk_b1, mk_b2)
    NK = len(subs)
    sub_cls = np.array([0 if s < mk_b1 else (1 if s < mk_b2 else 2) for s, _ in subs])
    incl_full = (sub_cls[None, :] != slab[:, None]).astype(np.float32)  # [B, NK]
    eye128 = np.eye(128, dtype=np.float32)

    # per-class embedding-sum vectors for the positives matmul (unscaled)
    gT = np.stack([emb[bord][slab == c].sum(axis=0) for c in range(C)], axis=1)
    gT = np.ascontiguousarray(gT).astype(ml_dtypes.bfloat16)

    # DVE pass A per-anchor shift: C0 = -inv_t/128 - u0
    cA = (KCBRT * (-inv_t / 128.0 - EXP_U0)).astype(np.float32)
    kcol = np.full((128, 1), EXP_K, dtype=np.float32)

    in_maps = []
    for core in range(NCORES):
        asl = slice(core * APC, (core + 1) * APC)
        oh = oneh[asl].reshape(NT, 128, C).transpose(1, 0, 2).reshape(128, NT * C)
        ic = incl_full[asl].reshape(NT, 128, NK).transpose(1, 0, 2).reshape(128, NT * NK)
        vecs = np.concatenate([
            _per_core_cols(inv_t, core),
            _per_core_cols(-inv_t, core),
            _per_core_cols(invpc, core),
            _per_core_cols(coefv, core),
            _per_core_cols(cA, core),
            oh.astype(np.float32),
            ic.astype(np.float32),
            kcol,
            eye128,
        ], axis=1)
        # scaled anchors: columns * inv_t_i/128 (scale BEFORE bf16 cast)
        anch_sc = (emb[bord][asl] * (inv_t[asl, None] / 128.0 * KCBRT)).T.astype(ml_dtypes.bfloat16)
        anch_un = embT[:, asl]
        in_maps.append({
            "embT": embT,
            "anchT": np.ascontiguousarray(
                np.concatenate([anch_sc, anch_un, gT], axis=1)),
            "bankT": bankT,
            "vecs": np.ascontiguousarray(vecs),
        })

    trace = os.environ.get("SUPCON_TRACE", "0") == "1"
    if trace:
        trace = _install_trace_shim()
    res = run_bass_kernel_spmd(nc, in_maps, core_ids=list(range(NCORES)), trace=trace)
    LAST_EXEC_TIME_NS = res.exec_time_ns

    # loss_i = coef_i * log(den_i) + lin_i; host finishes logs + masked mean
    loss_sum = np.float64(0.0)
    for core in range(NCORES):
        oo = np.asarray(res.results[core]["oout"], dtype=np.float64)    # [128, 2*NT]
        den, lin = oo[:, :NT], oo[:, NT:]
        cf = _per_core_cols(coefv, core).astype(np.float64)
        loss_sum += (cf * np.log(den) + lin).sum()
    return np.float32(loss_sum / max(n_valid, 1))
